# revision 1
# baseline (speedup 1.0000x reference)
"""AggregatedContrastiveLoss on 8 Trainium2 NeuronCores.

Strategy (data-parallel over the N=2M points dimension):
  - Each of 8 cores streams a ~250k-point shard of pred [N,128] f32 from HBM.
  - On device, per 128-point group, a one-hot selection matrix [128, 304] f16
    is built on VectorE from a host-packed key (key = seg + 152*group, or an
    out-of-range value for masked/overlap points), and TensorE accumulates
    predT @ onehot into PSUM [128d, 304] — the per-(class, group) feature
    sums for group A (cols 0..149) and group B (cols 152..301).
  - Host reduces the 8 partial [128,304] outputs, computes the per-class
    counts from the (tiny) int arrays, and runs the [150,128]-level
    normalize + [150,150] InfoNCE finalize in float64.
The `target` input is unused by the loss math and never transferred.
"""
import numpy as np

import concourse.bacc as bacc
import concourse.mybir as mybir
import concourse.tile as tile
from concourse.bass_utils import run_bass_kernel_spmd

F32 = mybir.dt.float32
F16 = mybir.dt.float16

N = 2_000_000
D = 128
C = 150
TEMPERATURE = 0.2
LOSS_WEIGHT = 1.0

N_CORES = 8
OWN = N // N_CORES            # 250_000 points owned per core
SHARD = 250_112               # 1954 groups of 128 (>= OWN, multiple of 128)
N_TILES = SHARD // 128        # 1954
CHUNK_TILES = 32              # 4096 points (2 MB f32) per DMA
W = 304                       # one-hot width (A: 0..149, B: 152..301)
BOFF = 152
INVALID = 1000.0

_STARTS = [min(i * OWN, N - SHARD) for i in range(N_CORES)]

_CHUNKS = []
_rem = N_TILES
while _rem > 0:
    _CHUNKS.append(min(CHUNK_TILES, _rem))
    _rem -= _CHUNKS[-1]


def _build_nc():
    nc = bacc.Bacc(
        "TRN2", target_bir_lowering=False, debug=False, num_devices=N_CORES
    )
    pred_d = nc.dram_tensor("pred", [SHARD, D], F32, kind="ExternalInput")
    key_d = nc.dram_tensor("key", [128, N_TILES], F32, kind="ExternalInput")
    iota_d = nc.dram_tensor("iota", [128, W], F16, kind="ExternalInput")
    out_d = nc.dram_tensor("out", [128, W], F32, kind="ExternalOutput")

    with tile.TileContext(nc) as tc:
        with (
            tc.tile_pool(name="io", bufs=3) as pio,
            tc.tile_pool(name="oh", bufs=8) as poh,
            tc.tile_pool(name="const", bufs=1) as pconst,
            tc.tile_pool(name="psum", bufs=1, space="PSUM") as pps,
        ):
            iota_sb = pconst.tile([128, W], F16)
            nc.sync.dma_start(iota_sb[:], iota_d[:])
            key_sb = pconst.tile([128, N_TILES], F32)
            nc.sync.dma_start(key_sb[:], key_d[:])
            acc = pps.tile([128, W], F32)

            t_idx = 0
            row = 0
            for ct in _CHUNKS:
                npts = ct * 128
                pf = pio.tile([128, npts], F32, tag="pf")
                src = pred_d[row : row + npts, :].rearrange(
                    "(p j) d -> p (j d)", p=128
                )
                nc.sync.dma_start(pf[:], src)
                ph = pio.tile([128, npts], F16, tag="ph")
                nc.scalar.copy(ph[:], pf[:])
                for j in range(ct):
                    oh = poh.tile([128, W], F16)
                    nc.vector.tensor_scalar(
                        oh[:],
                        iota_sb[:],
                        key_sb[:, t_idx : t_idx + 1],
                        None,
                        mybir.AluOpType.is_equal,
                    )
                    nc.tensor.matmul(
                        acc[:],
                        ph[:, j * 128 : (j + 1) * 128],
                        oh[:],
                        start=(t_idx == 0),
                        stop=(t_idx == N_TILES - 1),
                    )
                    t_idx += 1
                row += npts
            out_sb = pconst.tile([128, W], F32)
            nc.vector.tensor_copy(out_sb[:], acc[:])
            nc.sync.dma_start(out_d[:], out_sb[:])
    nc.compile()
    return nc


_NC = None


def _get_nc():
    global _NC
    if _NC is None:
        _NC = _build_nc()
    return _NC


def _key_layout(key_flat: np.ndarray) -> np.ndarray:
    """[SHARD] f32 -> [128, N_TILES] f32 matching the kernel's point order:
    within a chunk of `ct` groups starting at flat row `row`, partition p,
    column j holds point row + p*ct + j."""
    cols = []
    row = 0
    for ct in _CHUNKS:
        cols.append(key_flat[row : row + ct * 128].reshape(128, ct))
        row += ct * 128
    return np.ascontiguousarray(np.concatenate(cols, axis=1))


def kernel(pred, target, valid_feat_mask, segment, group_assign):
    pred = np.ascontiguousarray(np.asarray(pred, dtype=np.float32))
    seg = np.asarray(segment).astype(np.int64)
    grp = np.asarray(group_assign).astype(np.int64)
    vm = np.asarray(valid_feat_mask)

    valid = (vm > 0) & (seg != -1)
    segc = np.clip(seg, 0, C - 1)
    in_group = (grp == 0) | (grp == 1)
    key_full = np.where(
        valid & in_group, segc + BOFF * grp, int(INVALID)
    ).astype(np.float32)

    iota = np.tile(np.arange(W, dtype=np.float16), (128, 1))

    in_maps = []
    for i in range(N_CORES):
        s = _STARTS[i]
        k = key_full[s : s + SHARD].copy()
        own_lo, own_hi = i * OWN, (i + 1) * OWN
        gidx = np.arange(s, s + SHARD)
        k[(gidx < own_lo) | (gidx >= own_hi)] = INVALID
        in_maps.append(
            {
                "pred": pred[s : s + SHARD],
                "key": _key_layout(k),
                "iota": iota,
            }
        )

    nc = _get_nc()
    res = run_bass_kernel_spmd(nc, in_maps, core_ids=list(range(N_CORES)))

    total = np.zeros((128, W), np.float64)
    for r in res.results:
        total += r["out"].astype(np.float64)
    sum_a = total[:, 0:C].T          # [C, D]
    sum_b = total[:, BOFF : BOFF + C].T

    ga = valid & (grp == 0)
    gb = valid & (grp == 1)
    cnt_a = np.bincount(segc[ga], minlength=C).astype(np.float64)
    cnt_b = np.bincount(segc[gb], minlength=C).astype(np.float64)

    mean_a = sum_a / np.maximum(cnt_a, 1.0)[:, None]
    mean_b = sum_b / np.maximum(cnt_b, 1.0)[:, None]
    a = mean_a / np.linalg.norm(mean_a, axis=1, keepdims=True)
    b = mean_b / np.linalg.norm(mean_b, axis=1, keepdims=True)
    logits = (a @ b.T) / TEMPERATURE
    diag = np.diagonal(logits)

    def lse(x, axis):
        m = x.max(axis=axis)
        return m + np.log(np.exp(x - np.expand_dims(m, axis)).sum(axis=axis))

    loss_a = np.mean(lse(logits, 1) - diag)
    loss_b = np.mean(lse(logits, 0) - diag)
    loss = LOSS_WEIGHT * (loss_a + loss_b) / 2.0
    return np.asarray(loss, dtype=np.float32)


# revision 2
# speedup vs baseline: 1.0504x; 1.0504x over previous
"""AggregatedContrastiveLoss on 8 Trainium2 NeuronCores.

Strategy (data-parallel over the N=2M points dimension):
  - Each of 8 cores streams a ~250k-point shard of pred [N,128] f32 from HBM.
  - On device, per 128-point group, a one-hot selection matrix [128, 304] f16
    is built on VectorE from a host-packed key (key = seg + 152*group, or an
    out-of-range value for masked/overlap points), and TensorE accumulates
    predT @ onehot into PSUM [128d, 304] — the per-(class, group) feature
    sums for group A (cols 0..149) and group B (cols 152..301).
  - Host reduces the 8 partial [128,304] outputs, computes the per-class
    counts from the (tiny) int arrays, and runs the [150,128]-level
    normalize + [150,150] InfoNCE finalize in float64.
The `target` input is unused by the loss math and never transferred.
"""
import numpy as np

import concourse.bacc as bacc
import concourse.mybir as mybir
import concourse.tile as tile
from concourse.bass_utils import run_bass_kernel_spmd

F32 = mybir.dt.float32
F16 = mybir.dt.float16
AF = mybir.ActivationFunctionType

N = 2_000_000
D = 128
C = 150
TEMPERATURE = 0.2
LOSS_WEIGHT = 1.0

N_CORES = 8
OWN = N // N_CORES            # 250_000 points owned per core
SHARD = 250_112               # 1954 groups of 128 (>= OWN, multiple of 128)
N_TILES = SHARD // 128        # 1954
CHUNK_TILES = 64              # 8192 points (4 MB f32) per DMA
K_ACT = 5                     # one-hot groups per chunk built on ScalarE
W = 304                       # one-hot width (A: 0..149, B: 152..301)
BOFF = 152
INVALID = 1000.0

_STARTS = [min(i * OWN, N - SHARD) for i in range(N_CORES)]

_CHUNKS = []
_rem = N_TILES
while _rem > 0:
    _CHUNKS.append(min(CHUNK_TILES, _rem))
    _rem -= _CHUNKS[-1]


def _build_nc():
    nc = bacc.Bacc(
        "TRN2", target_bir_lowering=False, debug=False, num_devices=N_CORES
    )
    pred_d = nc.dram_tensor("pred", [SHARD, D], F32, kind="ExternalInput")
    key_d = nc.dram_tensor("key", [128, N_TILES], F32, kind="ExternalInput")
    nkey_d = nc.dram_tensor("nkey", [128, N_TILES], F32, kind="ExternalInput")
    iota_d = nc.dram_tensor("iota", [128, W], F16, kind="ExternalInput")
    out_d = nc.dram_tensor("out", [128, W], F32, kind="ExternalOutput")

    with tile.TileContext(nc) as tc:
        with (
            tc.tile_pool(name="io", bufs=3) as pio,
            tc.tile_pool(name="oh", bufs=12) as poh,
            tc.tile_pool(name="tmp", bufs=3) as ptmp,
            tc.tile_pool(name="const", bufs=1) as pconst,
            tc.tile_pool(name="psum", bufs=1, space="PSUM") as pps,
        ):
            iota_sb = pconst.tile([128, W], F16)
            nc.sync.dma_start(iota_sb[:], iota_d[:])
            key_sb = pconst.tile([128, N_TILES], F32)
            nc.sync.dma_start(key_sb[:], key_d[:])
            nkey_sb = pconst.tile([128, N_TILES], F32)
            nc.sync.dma_start(nkey_sb[:], nkey_d[:])
            acc = pps.tile([128, W], F32)

            t_idx = 0
            row = 0
            for ct in _CHUNKS:
                npts = ct * 128
                pf = pio.tile([128, npts], F32, tag="pf")
                src = pred_d[row : row + npts, :].rearrange(
                    "(p j) d -> p (j d)", p=128
                )
                nc.sync.dma_start(pf[:], src)
                ph = pio.tile([128, npts], F16, tag="ph")
                nc.scalar.copy(ph[:], pf[:])
                for j in range(ct):
                    oh = poh.tile([128, W], F16)
                    if j >= ct - K_ACT:
                        # ScalarE path: oh = relu(1 - |iota - key|), exact
                        tmp = ptmp.tile([128, W], F16)
                        nc.scalar.activation(
                            tmp[:], iota_sb[:], AF.Abs,
                            bias=nkey_sb[:, t_idx : t_idx + 1], scale=1.0,
                        )
                        nc.scalar.activation(
                            oh[:], tmp[:], AF.Relu, bias=1.0, scale=-1.0,
                        )
                    else:
                        nc.vector.tensor_scalar(
                            oh[:],
                            iota_sb[:],
                            key_sb[:, t_idx : t_idx + 1],
                            None,
                            mybir.AluOpType.is_equal,
                        )
                    nc.tensor.matmul(
                        acc[:],
                        ph[:, j * 128 : (j + 1) * 128],
                        oh[:],
                        start=(t_idx == 0),
                        stop=(t_idx == N_TILES - 1),
                    )
                    t_idx += 1
                row += npts
            out_sb = pconst.tile([128, W], F32)
            nc.vector.tensor_copy(out_sb[:], acc[:])
            nc.sync.dma_start(out_d[:], out_sb[:])
    nc.compile()
    return nc


_NC = None


def _get_nc():
    global _NC
    if _NC is None:
        _NC = _build_nc()
    return _NC


def _key_layout(key_flat: np.ndarray) -> np.ndarray:
    """[SHARD] f32 -> [128, N_TILES] f32 matching the kernel's point order:
    within a chunk of `ct` groups starting at flat row `row`, partition p,
    column j holds point row + p*ct + j."""
    cols = []
    row = 0
    for ct in _CHUNKS:
        cols.append(key_flat[row : row + ct * 128].reshape(128, ct))
        row += ct * 128
    return np.ascontiguousarray(np.concatenate(cols, axis=1))


def kernel(pred, target, valid_feat_mask, segment, group_assign):
    pred = np.ascontiguousarray(np.asarray(pred, dtype=np.float32))
    seg = np.asarray(segment).astype(np.int64)
    grp = np.asarray(group_assign).astype(np.int64)
    vm = np.asarray(valid_feat_mask)

    valid = (vm > 0) & (seg != -1)
    segc = np.clip(seg, 0, C - 1)
    in_group = (grp == 0) | (grp == 1)
    key_full = np.where(
        valid & in_group, segc + BOFF * grp, int(INVALID)
    ).astype(np.float32)

    iota = np.tile(np.arange(W, dtype=np.float16), (128, 1))

    in_maps = []
    for i in range(N_CORES):
        s = _STARTS[i]
        k = key_full[s : s + SHARD].copy()
        own_lo, own_hi = i * OWN, (i + 1) * OWN
        gidx = np.arange(s, s + SHARD)
        k[(gidx < own_lo) | (gidx >= own_hi)] = INVALID
        in_maps.append(
            {
                "pred": pred[s : s + SHARD],
                "key": (k2 := _key_layout(k)),
                "nkey": -k2,
                "iota": iota,
            }
        )

    nc = _get_nc()
    res = run_bass_kernel_spmd(nc, in_maps, core_ids=list(range(N_CORES)))

    total = np.zeros((128, W), np.float64)
    for r in res.results:
        total += r["out"].astype(np.float64)
    sum_a = total[:, 0:C].T          # [C, D]
    sum_b = total[:, BOFF : BOFF + C].T

    ga = valid & (grp == 0)
    gb = valid & (grp == 1)
    cnt_a = np.bincount(segc[ga], minlength=C).astype(np.float64)
    cnt_b = np.bincount(segc[gb], minlength=C).astype(np.float64)

    mean_a = sum_a / np.maximum(cnt_a, 1.0)[:, None]
    mean_b = sum_b / np.maximum(cnt_b, 1.0)[:, None]
    a = mean_a / np.linalg.norm(mean_a, axis=1, keepdims=True)
    b = mean_b / np.linalg.norm(mean_b, axis=1, keepdims=True)
    logits = (a @ b.T) / TEMPERATURE
    diag = np.diagonal(logits)

    def lse(x, axis):
        m = x.max(axis=axis)
        return m + np.log(np.exp(x - np.expand_dims(m, axis)).sum(axis=axis))

    loss_a = np.mean(lse(logits, 1) - diag)
    loss_b = np.mean(lse(logits, 0) - diag)
    loss = LOSS_WEIGHT * (loss_a + loss_b) / 2.0
    return np.asarray(loss, dtype=np.float32)
